# revision 1
# baseline (speedup 1.0000x reference)
"""BERT-CRF Viterbi decode kernel for Trainium2 (Bass/Tile), 8-core data parallel.

alpha+beta formulation with host-pretransposed fp16 activations.

Full inputs in, full outputs out. Batch B=64 sharded across 8 cores (8 seqs
each). Per core, scan rows r = b*16 + c (c = chunk of L=32 timesteps).

  Host: sentences cast to fp16 and pre-transposed to [p=h%128, (g, ch, uu,
  row)] so the PE matmul rhs streams straight out of DMA (no on-device
  transposes, half the HBM bytes of fp32). W is split fp16 hi+lo and packed
  on the lhsT M axis (hi at out-partitions 0:4, lo at 32:36), so a single
  rhs stream yields S16*W16hi and S16*W16lo; summing the two transposed
  quads restores ~fp32-grade emissions (measured rel err ~1e-2 on tags,
  limit 2e-2).

  Stage A (per group g of 4 steps): one 786KB DMA (triggers spread over the
  two HWDGE rings), 6 fp16 accumulating matmuls -> e^T [36,512] PSUM, 8
  cheap PE transposes (hi/lo), hi+lo add -> emissions; then (Vector)
  T'_u = trans+b+e_u, the forward max-plus recurrence Apre_u, and the
  backward local suffix matrices Lv/F per group. All overlapped with the
  ~21us DMA stream.

  Tail: suffix-of-groups scan Ss, backward boundary scan p2b, forward
  boundary scan p2f (all Vector; GpSimd/Pool has no usable tensor ops in
  this build), then beta = Lv o Ss o bb, alpha = sb o Apre, and
  tags = first-argmax_j(alpha_u[j] + beta_u[j]) -- no backpointers, no
  backtracking, no serial 512-step scan anywhere.
"""
import sys
for p in ("/opt/trn_rl_repo", "/root/.axon_site/_ro/trn_rl_repo"):
    if p not in sys.path:
        sys.path.append(p)

import numpy as np
import concourse.bass as bass
import concourse.tile as tile
from concourse import mybir
from concourse.bass_utils import run_bass_kernel_spmd

F32 = mybir.dt.float32
F32R = mybir.dt.float32r
BF16 = mybir.dt.bfloat16
FP16 = mybir.dt.float16
I32 = mybir.dt.int32
AX = mybir.AxisListType
OP = mybir.AluOpType

B, T, H, K = 64, 512, 768, 4
NCORES = 8
BC = B // NCORES          # 8 sequences per core
C, L = 16, 32             # chunks per sequence, steps per chunk
ROWS = BC * C             # 128 partition rows
HCH = H // 128            # 6 h-chunks
UG = 4                    # steps per group
NG = L // UG              # 8 groups
GW = HCH * UG * 128       # 3072 fp16 cols per partition per group

NEG = -1.0e30

_NC_CACHE = {}


def build_nc():
    nc = bass.Bass()
    sentd = nc.declare_dram_parameter("sentT", [128, NG * GW], FP16, isOutput=False)
    wtd = nc.declare_dram_parameter("wt", [128, HCH * 36], FP16, isOutput=False)
    # rowconsts[128, 64]: wfirst | iw | mpid | end | tbT | tbinitT | id4
    rcd = nc.declare_dram_parameter("rowconsts", [128, 64], F32, isOutput=False)
    tagsd = nc.declare_dram_parameter("tags", [BC, T], I32, isOutput=True)

    with tile.TileContext(nc) as tc:
        with tc.tile_pool(name="singles", bufs=1) as singles, \
             tc.tile_pool(name="gpool", bufs=NG) as gpool, \
             tc.tile_pool(name="et_pool", bufs=3) as et_pool, \
             tc.tile_pool(name="tmp_pool", bufs=2) as tmp_pool, \
             tc.tile_pool(name="gtmp_pool", bufs=2) as gtmp_pool, \
             tc.tile_pool(name="ps_eT", bufs=3, space="PSUM") as ps_eT, \
             tc.tile_pool(name="ps_fix", bufs=2, space="PSUM") as ps_fix:

            # ---------- constants (on the Act ring; Sync ring leads with
            # the first sentence piece so the PE starts earliest) ----------
            wt = singles.tile([128, HCH * 36], FP16)
            nc.scalar.dma_start(wt, wtd[:])
            rc = singles.tile([128, 64], F32)
            nc.scalar.dma_start(rc, rcd[:])
            wfirst = rc[:, 0:4]
            iw4 = rc[:, 4:8]
            mpid = rc[:, 8:24]          # max-plus identity (0 diag, NEG off)
            end_sb = rc[:, 24:28]
            tbT = rc[:, 28:44]          # tbT[j*4+k] = trans[k,j] + b[j]
            tbinitT = rc[:, 44:60]      # same with tinit (start row c==0)
            id84 = rc[:, 60:64]         # stacked [I4; I4] on partitions 0..7

            # ---------- prefetch all sentence groups ----------
            # Group 0 is split into sub-DMAs so the first matmul starts as
            # soon as its first h-chunks land instead of after the full 1.5MB.
            # Triggers are spread across the two HWDGE rings (Sync=SP and
            # Scalar=Act) -- each trigger costs ~650ns of issue time serially
            # per ring.
            gtiles = []
            for g in range(NG):
                gt = gpool.tile([128, GW], FP16, tag="gt")
                if g == 0:
                    for c0, c1 in ((0, 1024), (1024, 2048), (2048, GW)):
                        src = bass.AP(tensor=sentd[:].tensor, offset=c0,
                                      ap=[[NG * GW, 128], [1, c1 - c0]])
                        nc.sync.dma_start(gt[:, c0:c1], src)
                else:
                    src = bass.AP(tensor=sentd[:].tensor, offset=g * GW,
                                  ap=[[NG * GW, 128], [1, GW]])
                    eng = nc.sync if g < 4 else nc.scalar
                    eng.dma_start(gt, src)
                gtiles.append(gt)

            # ---------- persistent state ----------
            emsc = singles.tile([128, L * K], F32)
            emv = emsc.rearrange("p (u j) -> p u j", u=L)
            # T'^T storage: TT[u][j][k] = trans[k,j] + b[j] + e_u[j]
            TT = singles.tile([128, L, 4, 4], F32)
            # forward prefix mats Apre[u][i][j]
            Apre = singles.tile([128, L, 4, 4], F32)
            # backward local suffixes: LT[g][v][i][x] = Lv_{g,v}[i][x]
            LT = singles.tile([128, NG, UG, 4, 4], F32)
            # full-group transfers: FT[g][i][x] = F_g[i][x]
            FT = singles.tile([128, NG, 4, 4], F32)
            # suffix-of-groups, transposed: SsT[g][z][a] = Ss_g[a][z]
            SsT = singles.tile([128, NG, 4, 4], F32)

            # LT[g][3] = max-plus identity, all groups at once
            nc.vector.tensor_copy(
                LT[:, :, 3, :, :],
                mpid.rearrange("p (x i) -> p x i", x=4)
                    .unsqueeze(1).to_broadcast((128, NG, 4, 4)))

            # ---------- Stage A ----------
            sA = nc.named_scope("stageA")
            sA.__enter__()
            for g in range(NG):
                gt = gtiles[g]
                gv = gt.rearrange("p (ch n) -> p ch n", ch=HCH)
                # eT rows 0-3: S16*W16hi; rows 32-35: S16*W16lo.  The lo
                # block sits at partition 32 so the transpose lhsT reads are
                # quad-aligned (base partition must be 0/32/64).
                eT_ps = ps_eT.tile([36, UG * 128], F32, tag="eT")
                for ch in range(HCH):
                    nc.tensor.matmul(
                        eT_ps,
                        wt[:, ch * 36:(ch + 1) * 36],   # Whi @0:4, Wlo @32:36
                        gv[:, ch, :],
                        start=(ch == 0), stop=(ch == HCH - 1))
                eT_sb = et_pool.tile([36, UG * 128], F32, tag="eTsb")
                nc.scalar.copy(eT_sb, eT_ps)
                fxh = ps_fix.tile([128, UG * K], F32, tag="fixh")
                fxl = ps_fix.tile([128, UG * K], F32, tag="fixl")
                for uu in range(UG):
                    nc.tensor.transpose(
                        fxh[:, uu * 4:(uu + 1) * 4],
                        eT_sb[0:4, uu * 128:(uu + 1) * 128], id84[0:4, :])
                    nc.tensor.transpose(
                        fxl[:, uu * 4:(uu + 1) * 4],
                        eT_sb[32:36, uu * 128:(uu + 1) * 128], id84[32:36, :])
                # e = hi + lo (only one PSUM operand allowed per DVE op)
                ehs = et_pool.tile([128, UG * K], F32, tag="ehs")
                nc.scalar.copy(ehs, fxh)
                nc.vector.tensor_tensor(
                    emsc[:, g * UG * K:(g + 1) * UG * K], ehs, fxl, OP.add)
                # ---- T'^T for this group: TT[u][j][k] = e_u[j] + tbT[j,k]
                if g == 0:
                    nc.vector.tensor_tensor(
                        TT[:, 0, :, :],
                        emv[:, 0, :].unsqueeze(2).to_broadcast((128, 4, 4)),
                        tbinitT.rearrange("p (j k) -> p j k", j=4),
                        OP.add)
                    nc.vector.tensor_tensor(
                        TT[:, 1:UG, :, :],
                        emv[:, 1:UG, :].unsqueeze(3).to_broadcast((128, UG - 1, 4, 4)),
                        tbT.rearrange("p (j k) -> p j k", j=4)
                           .unsqueeze(1).to_broadcast((128, UG - 1, 4, 4)),
                        OP.add)
                else:
                    nc.vector.tensor_tensor(
                        TT[:, g * UG:(g + 1) * UG, :, :],
                        emv[:, g * UG:(g + 1) * UG, :]
                            .unsqueeze(3).to_broadcast((128, UG, 4, 4)),
                        tbT.rearrange("p (j k) -> p j k", j=4)
                           .unsqueeze(1).to_broadcast((128, UG, 4, 4)),
                        OP.add)
                # ---- forward recurrence (Vector)
                for uu in range(UG):
                    u = g * UG + uu
                    if u == 0:
                        nc.scalar.copy(
                            Apre[:, 0, :, :], TT[:, 0, :, :].transpose([0, 2, 1]))
                    else:
                        t4 = tmp_pool.tile([128, 4, 4, 4], F32, tag="fwd")
                        # t4[i,j,k] = Apre[u-1][i,k] + TT[u][j,k]
                        nc.vector.tensor_tensor(
                            t4,
                            Apre[:, u - 1, :, :].unsqueeze(2).to_broadcast((128, 4, 4, 4)),
                            TT[:, u, :, :].unsqueeze(1).to_broadcast((128, 4, 4, 4)),
                            OP.add)
                        nc.vector.reduce_max(Apre[:, u, :, :], t4, axis=AX.X)
                # ---- backward local suffixes (Vector compose; Scalar copy)
                def bw_compose(out_ix, prev_ix, TTu_jk):
                    # out[i][x] = max_j T'[i][j] + prev[j][x]
                    tb4 = gtmp_pool.tile([128, 4, 4, 4], F32, tag="bwd")
                    nc.vector.tensor_tensor(
                        tb4,
                        prev_ix.transpose([0, 2, 1])      # [x, j]
                               .unsqueeze(1).to_broadcast((128, 4, 4, 4)),
                        TTu_jk.transpose([0, 2, 1])       # [i, j] (= T')
                              .unsqueeze(2).to_broadcast((128, 4, 4, 4)),
                        OP.add)                            # tb4[i, x, j]
                    nc.vector.reduce_max(out_ix, tb4, axis=AX.X)

                t3 = g * UG + 3
                nc.scalar.copy(
                    LT[:, g, 2, :, :], TT[:, t3, :, :].transpose([0, 2, 1]))
                for v in (1, 0):
                    bw_compose(LT[:, g, v, :, :], LT[:, g, v + 1, :, :],
                               TT[:, g * UG + v + 1, :, :])
                if g > 0:
                    bw_compose(FT[:, g, :, :], LT[:, g, 0, :, :],
                               TT[:, g * UG, :, :])
            sA.__exit__(None, None, None)

            # ---------- Ss: suffix-of-groups scan (Vector) ----------
            _sss = nc.named_scope("ssscan")
            _sss.__enter__()
            nc.vector.tensor_copy(
                SsT[:, NG - 1, :, :], mpid.rearrange("p (x i) -> p x i", x=4))
            for g in range(NG - 2, -1, -1):
                # SsT[g][z][a] = max_m SsT[g+1][z][m] + F_{g+1}[a][m]
                t4 = tmp_pool.tile([128, 4, 4, 4], F32, tag="fwd")
                nc.vector.tensor_tensor(
                    t4,
                    SsT[:, g + 1, :, :].unsqueeze(2).to_broadcast((128, 4, 4, 4)),
                    FT[:, g + 1, :, :]                 # [a, m]
                      .unsqueeze(1).to_broadcast((128, 4, 4, 4)),
                    OP.add)
                nc.vector.reduce_max(SsT[:, g, :, :], t4, axis=AX.X)
            _sss.__exit__(None, None, None)

            # ---------- regroup A_c to by-b layout ----------
            # Split so the high-c half (consumed first by p2b) lands first.
            _sp2 = nc.named_scope("p2")
            _sp2.__enter__()
            abyb = singles.tile([BC, C * 16], F32)
            nc.sync.dma_start(abyb, Apre[:, L - 1, :, :].rearrange("p a b -> p (a b)"))
            abv = abyb.rearrange("p (c i j) -> p c i j", c=C, i=4)

            # ----- p2b: backward boundary scores, rows 0..7 -----
            bby = singles.tile([BC, C * 4], F32)
            bbv = bby.rearrange("p (c j) -> p c j", c=C)
            nc.scalar.copy(bbv[:, C - 1, :], end_sb[0:BC, :])
            for c in range(C - 1, 0, -1):
                # bb_{c-1}[i] = max_j (A_c[i,j] + bb_c[j])
                p2tmp = gtmp_pool.tile([BC, 4, 4], F32, tag="p2b")
                nc.vector.tensor_tensor(
                    p2tmp,
                    abv[:, c, :, :],
                    bbv[:, c, :].unsqueeze(1).to_broadcast((BC, 4, 4)),
                    OP.add)
                nc.vector.reduce_max(bbv[:, c - 1, :], p2tmp, axis=AX.X)
            # broadcast bb to rows: bbc[128, 4], row b*16+c = bb_c[b]
            bbc = singles.tile([128, 4], F32)
            nc.sync.dma_start(bbc, bby)

            # ----- p2f: forward boundary scores, rows 0..7 -----
            sbound = singles.tile([BC, (C + 1) * 4], F32)
            nc.vector.memset(sbound[:, 0:4], 0.0)
            sbv = sbound.rearrange("p (c j) -> p c j", c=C + 1)
            for c in range(C):
                p2tmp = tmp_pool.tile([BC, 4, 4], F32, tag="p2f")
                # tmp[j,i] = s[i] + A_c[i,j]
                nc.vector.tensor_tensor(
                    p2tmp,
                    sbv[:, c, :].unsqueeze(1).to_broadcast((BC, 4, 4)),
                    abv[:, c, :, :].transpose([0, 2, 1]),
                    OP.add)
                nc.vector.reduce_max(sbv[:, c + 1, :], p2tmp, axis=AX.X)
            _sp2.__exit__(None, None, None)

            # ---------- beta, then alpha (beta overlaps the scores DMA) ----
            _sp3 = nc.named_scope("p3")
            _sp3.__enter__()
            scores = singles.tile([128, (L + 1) * 4], F32)
            nc.sync.dma_start(scores[:, 0:4], sbound[:, 0:C * 4])
            scv = scores.rearrange("p (u i) -> p u i", u=L + 1)
            # bsub[g][a] = max_z Ss_g[a][z] + bb_row[z]
            bsub = singles.tile([128, NG, 4], F32)
            bst = gtmp_pool.tile([128, NG, 4, 4], F32, tag="bst")
            nc.vector.tensor_tensor(
                bst,
                SsT.transpose([0, 1, 3, 2]),
                bbc.unsqueeze(1).unsqueeze(1).to_broadcast((128, NG, 4, 4)),
                OP.add)
            nc.vector.reduce_max(bsub, bst, axis=AX.X)
            # beta[u=(g,v)][i] = max_x Lv_{g,v}[i][x] + bsub[g][x]
            # LT viewed [p, g, (v i), x] keeps ops within 3 free dims.
            beta = singles.tile([128, L * 4], F32)
            bev = beta.rearrange("p (g vi) -> p g vi", g=NG)   # vi = v*4+i
            beuv = beta.rearrange("p (u i) -> p u i", u=L)
            LTm = LT.rearrange("p g v i x -> p g (v i) x")
            VI = UG * 4
            btmp = singles.tile([128, NG, VI, 4], F32)
            nc.vector.tensor_tensor(
                btmp,
                LTm,
                bsub.unsqueeze(2).to_broadcast((128, NG, VI, 4)),
                OP.add)
            nc.vector.reduce_max(bev, btmp, axis=AX.X)
            # alpha for all steps
            p3tmp = singles.tile([128, L, 4, 4], F32)   # [u, j, i]
            nc.vector.tensor_tensor(
                p3tmp,
                scores[:, 0:4].unsqueeze(1).unsqueeze(1).to_broadcast((128, L, 4, 4)),
                Apre.transpose([0, 1, 3, 2]),
                OP.add)
            nc.vector.reduce_max(scv[:, 1:, :], p3tmp, axis=AX.X)
            _sp3.__exit__(None, None, None)

            # ---------- tags: first-argmax_j(alpha+beta) ----------
            _sp5 = nc.named_scope("p5")
            _sp5.__enter__()
            # tag = first-argmax_j of delta (wfirst weighting breaks ties
            # toward the smallest j, matching jnp.argmax)
            delta = singles.tile([128, L, 4], F32)
            nc.vector.tensor_tensor(delta, scv[:, 1:, :], beuv, OP.add)
            mx = tmp_pool.tile([128, L], F32, tag="mx")
            nc.vector.reduce_max(mx, delta, axis=AX.X)
            eq = singles.tile([128, L, 4], F32)
            nc.vector.tensor_tensor(
                eq, delta, mx.unsqueeze(2).to_broadcast((128, L, 4)), OP.is_equal)
            nc.vector.tensor_tensor(
                eq, eq, wfirst.unsqueeze(1).to_broadcast((128, L, 4)), OP.mult)
            nc.vector.reduce_max(mx, eq, axis=AX.X)
            nc.vector.tensor_tensor(
                eq, eq, mx.unsqueeze(2).to_broadcast((128, L, 4)), OP.is_equal)
            nc.vector.tensor_tensor(
                eq, eq, iw4.unsqueeze(1).to_broadcast((128, L, 4)), OP.mult)
            tagf = tmp_pool.tile([128, L], F32, tag="tagf")
            nc.vector.reduce_sum(tagf, eq, axis=AX.X)
            tagi = tmp_pool.tile([128, L], I32, tag="tagi")
            nc.scalar.copy(tagi, tagf)
            nc.scalar.dma_start(tagsd[:].rearrange("b (c t) -> b c t", c=C), tagi)
            _sp5.__exit__(None, None, None)

    return nc


def _split_multi_waits(nc, waits_per_drain=1):
    """Walrus (bass2jax path) allows very few embedded sync waits per
    instruction (PE matmul: exactly 1). Hoist multi-waits onto standalone
    InstDrain instructions on the same engine, preserving order."""
    for f in nc.m.functions:
        for blk in f.blocks:
            insts = blk.instructions
            i = 0
            while i < len(insts):
                ins = insts[i]
                si = ins.sync_info
                w = list(si.on_wait) if (si is not None and si.on_wait) else []
                if len(w) >= 2:
                    groups = [w[j:j + waits_per_drain]
                              for j in range(0, len(w), waits_per_drain)]
                    for k, grp in enumerate(groups):
                        d = mybir.InstEventSemaphore(
                            name=nc.get_next_instruction_name(), ins=[], outs=[])
                        d.engine = ins.engine
                        d.sync_info = mybir.SyncInfo(on_wait=grp, on_update=[])
                        insts.insert(i + k, d)
                    i += len(groups)
                    ins.sync_info = mybir.SyncInfo(
                        on_wait=[], on_update=list(si.on_update or []))
                i += 1


def _get_nc():
    if "nc" not in _NC_CACHE:
        nc = build_nc()
        _split_multi_waits(nc, waits_per_drain=1)   # HW path only; CoreSim rejects raw drains
        _NC_CACHE["nc"] = nc
    return _NC_CACHE["nc"]


def make_in_maps(inputs):
    sent = np.ascontiguousarray(np.asarray(inputs["sentences"], dtype=np.float32))
    W = np.ascontiguousarray(np.asarray(inputs["W"], dtype=np.float32))
    bb = np.asarray(inputs["b"], dtype=np.float32)
    st = np.asarray(inputs["start_transitions"], dtype=np.float32)
    en = np.asarray(inputs["end_transitions"], dtype=np.float32)
    tr = np.asarray(inputs["transitions"], dtype=np.float32)

    # W^T chunks, fp16 hi/lo packed on M: W16hi at cols 0:4, W16lo at 32:36
    wT = np.transpose(W.reshape(K, HCH, 128), (2, 1, 0))   # [p, ch, k]
    whi = wT.astype(np.float16).astype(np.float32)
    wlo = (wT - whi).astype(np.float16).astype(np.float32)
    wt = np.zeros((128, HCH, 36), dtype=np.float32)
    wt[:, :, 0:4] = whi
    wt[:, :, 32:36] = wlo
    wt = np.ascontiguousarray(wt.reshape(128, HCH * 36)).astype(np.float16)

    # tbT[j*4+k] = trans[k,j] + b[j]
    tbT = (tr.T + bb[:, None]).reshape(16).astype(np.float32)  # [j,k] row-major
    # tinit rows: c==0 -> start (indep of k), else trans
    tinitT = np.tile(tbT, (128, 1))
    tbinit0 = (np.tile(st[:, None], (1, 4)) + bb[:, None]).reshape(16)
    tinitT[0::C, :] = tbinit0[None, :]

    mpid = (np.where(np.eye(4, dtype=bool), 0.0, NEG)).astype(np.float32).ravel()

    rc = np.zeros((128, 64), dtype=np.float32)
    rc[:, 0:4] = [4.0, 3.0, 2.0, 1.0]
    rc[:, 4:8] = [0.0, 1.0, 2.0, 3.0]
    rc[:, 8:24] = mpid[None, :]
    rc[:, 24:28] = en[None, :]
    rc[:, 28:44] = tbT[None, :]
    rc[:, 44:60] = tinitT
    rc[0:4, 60:64] = np.eye(4, dtype=np.float32)
    rc[32:36, 60:64] = np.eye(4, dtype=np.float32)

    in_maps = []
    for core in range(NCORES):
        sc = sent[core * BC:(core + 1) * BC]           # [8, 512, 768]
        s6 = sc.reshape(BC, C, NG, UG, HCH, 128)       # b c g uu ch p
        sT = np.transpose(s6, (5, 2, 4, 3, 0, 1))      # p g ch uu b c
        sT = np.ascontiguousarray(sT.reshape(128, NG * GW)).astype(np.float16)
        in_maps.append({
            "sentT": sT, "wt": wt, "rowconsts": rc,
        })
    return in_maps


def kernel(**inputs):
    nc = _get_nc()
    in_maps = make_in_maps(inputs)
    res = run_bass_kernel_spmd(nc, in_maps, core_ids=list(range(NCORES)))
    tags = np.concatenate([res.results[c]["tags"] for c in range(NCORES)], axis=0)
    return tags.astype(np.int32)


if __name__ == "__main__":
    import reference
    inputs = {k: np.asarray(v) for k, v in reference.setup_inputs().items()}
    out = kernel(**inputs)
    print(out.shape, out.dtype, out[:2, :16])



# revision 6
# speedup vs baseline: 1.0659x; 1.0659x over previous
"""BERT-CRF Viterbi decode kernel for Trainium2 (Bass/Tile), 8-core data parallel.

v2: transitions folded into the PE matmul; batched group-level max-plus algebra.

Full inputs in, full outputs out. Batch B=64 sharded across 8 cores (8 seqs
each). Per core, 128 partition rows = (b=8 seqs) x (c=16 chunks of L=32 steps);
each chunk splits into NG=8 groups of UG=4 steps.

  Host pre-transposes sentences to fp16 [p=h%128, (g, ch, uu, row)] and packs
  W as fp16 hi+lo replicated over the next-tag axis k, so one PE pass per
  group produces eT[(j,k), (uu,row)] = trans[k,j]+b[j]+e_u[j] directly (the
  trans+b term rides a tiny hi/lo seed matmul against a ones-row). PE
  transposes deliver T^T per row; one DVE add (hi+lo) finishes the fp32-grade
  step matrices TS[u][j][k]. No separate emission/broadcast stage.

  Per group (hidden under the ~21us DMA stream): intra-group prefix mats
  PT[g,uu] (3 composes), exclusive suffix mats SN[g,uu] (2 composes + 1
  scalar-copy), and the running cross-group prefix GpreT[g] (1 compose) --
  13 small Vector ops per group vs ~28 in v1.

  Tail: exclusive suffix-of-groups GsufN (7 composes), chunk totals M ->
  per-sequence boundary scans p2f/p2b over 16 chunks (8 rows), then batched
  boundary->group (asb/bsb), group->step backfill (alpha/beta), and
  tags = first-argmax_j(alpha_u[j] + beta_u[j]). No backpointers, no serial
  512-step scan anywhere.
"""
import sys
for p in ("/opt/trn_rl_repo", "/root/.axon_site/_ro/trn_rl_repo"):
    if p not in sys.path:
        sys.path.append(p)

import numpy as np
import concourse.bass as bass
import concourse.tile as tile
from concourse import mybir
from concourse.bass_utils import run_bass_kernel_spmd

F32 = mybir.dt.float32
FP16 = mybir.dt.float16
I32 = mybir.dt.int32
AX = mybir.AxisListType
OP = mybir.AluOpType

B, T, H, K = 64, 512, 768, 4
NCORES = 8
BC = B // NCORES          # 8 sequences per core
C, L = 16, 32             # chunks per sequence, steps per chunk
ROWS = BC * C             # 128 partition rows
HCH = H // 128            # 6 h-chunks
UG = 4                    # steps per group
NG = L // UG              # 8 groups
GW = HCH * UG * 128       # 3072 fp16 cols per partition per group
WCOLS = (HCH + 1) * 48    # W' lhsT cols: 6 chunks x 48 + seed block

NEG = -1.0e30

_NC_CACHE = {}


def build_nc():
    nc = bass.Bass()
    sentd = nc.declare_dram_parameter("sentT", [128, NG * GW], FP16, isOutput=False)
    wtd = nc.declare_dram_parameter("wt", [128, WCOLS], FP16, isOutput=False)
    # rowconsts[128, 128]: wfirst | iw4 | mpid | end | rcfix | id48
    rcd = nc.declare_dram_parameter("rowconsts", [128, 128], F32, isOutput=False)
    tagsd = nc.declare_dram_parameter("tags", [BC, T], I32, isOutput=True)

    with tile.TileContext(nc) as tc:
        with tc.tile_pool(name="singles", bufs=1) as singles, \
             tc.tile_pool(name="gpool", bufs=NG) as gpool, \
             tc.tile_pool(name="et_pool", bufs=3) as et_pool, \
             tc.tile_pool(name="tmp_pool", bufs=2) as tmp_pool, \
             tc.tile_pool(name="gtmp_pool", bufs=2) as gtmp_pool, \
             tc.tile_pool(name="ps_eT", bufs=2, space="PSUM") as ps_eT, \
             tc.tile_pool(name="ps_fx", bufs=2, space="PSUM") as ps_fx:

            # ---------- constants (Act ring; Sync ring leads with group 0) ----
            wt = singles.tile([128, WCOLS], FP16)
            nc.scalar.dma_start(wt, wtd[:])
            rc = singles.tile([128, 128], F32)
            nc.scalar.dma_start(rc, rcd[:])
            wfirst = rc[:, 0:4]
            iw4 = rc[:, 4:8]
            mpid = rc[:, 8:24]          # max-plus identity (0 diag, NEG off)
            end_sb = rc[:, 24:28]
            rcfix = rc[:, 28:44]        # rows c==0: start[j]-trans[k,j]; else 0
            id48 = rc[0:48, 44:92]

            # ---------- prefetch all sentence groups ----------
            gtiles = []
            for g in range(NG):
                gt = gpool.tile([128, GW], FP16, tag="gt")
                if g == 0:
                    for c0, c1 in ((0, 1024), (1024, 2048), (2048, GW)):
                        src = bass.AP(tensor=sentd[:].tensor, offset=c0,
                                      ap=[[NG * GW, 128], [1, c1 - c0]])
                        nc.sync.dma_start(gt[:, c0:c1], src)
                else:
                    src = bass.AP(tensor=sentd[:].tensor, offset=g * GW,
                                  ap=[[NG * GW, 128], [1, GW]])
                    eng = nc.sync if g < 4 else nc.scalar
                    eng.dma_start(gt, src)
                gtiles.append(gt)

            # ---------- persistent state ----------
            # TS[g][uu][j*4+k] = trans[k,j] + b[j] + e_u[j]   (T_u transposed)
            TS = singles.tile([128, NG, UG, 16], F32)
            # PT[g][s][j*4+i] = P[g,s+1][i][j]  (intra-group inclusive prefix,
            # transposed; s=0..2 for uu=1..3; uu=0 is TS[g,0] itself)
            PT = singles.tile([128, NG, 3, 16], F32)
            # SN[g][s][i*4+x] = exclusive suffix T_{g,s+1}o..oT_{g,3} (natural)
            SN = singles.tile([128, NG, 3, 16], F32)
            # GpreT[g][j*4+i] = exclusive prefix-of-groups (transposed); [NG]=M
            GpreT = singles.tile([128, NG + 1, 16], F32)
            # GsufN[g][i*4+x] = exclusive suffix-of-groups (natural)
            GsufN = singles.tile([128, NG, 16], F32)
            # asb/bsb: boundary vectors at group granularity
            ab = singles.tile([128, 2, NG, 4], F32)
            alpha = singles.tile([128, NG, UG, 4], F32)
            beta = singles.tile([128, NG, UG, 4], F32)
            # p2 scan state: [0, s] = fwd alpha s_0..s_16; [1, c] = bwd bb_c
            states = singles.tile([BC, 2, C + 1, 4], F32)
            # ab2[0][c][j*4+i] = M_c transposed; ab2[1][c][i*4+j] = M_c natural
            ab2 = singles.tile([BC, 2, C, 16], F32)
            sbc = singles.tile([128, 8], F32)   # cols 0:4 = sb_c, 4:8 = bb_c

            # prefills
            nc.vector.tensor_copy(GpreT[:, 0, :], mpid)
            nc.vector.tensor_copy(GsufN[:, NG - 1, :], mpid)
            nc.vector.memset(states[:, 0, 0, :], 0.0)
            nc.scalar.copy(states[:, 1, C - 1, :], end_sb[0:BC, :])
            ones = singles.tile([2, UG * 128], FP16)
            nc.vector.memset(ones, 1.0)

            def mm_compose(out_ji, lhs_jm, rhs_mi, pool, tag):
                """out[j][i] = max_m lhs[j][m] + rhs[m][i].

                lhs_jm, rhs_mi, out_ji: AP views shaped [128, ..., 4, 4] with
                matching batch dims; lhs indexed [.., j, m], rhs [.., m, i].
                """
                nb = lhs_jm.shape[1:-2]
                shp = (128, *nb, 4, 4, 4)
                t4 = pool.tile(list(shp), F32, tag=tag)
                nd = len(shp)
                # in0[.., j, i, m] = lhs[.., j, m]
                in0 = lhs_jm.unsqueeze(nd - 2).to_broadcast(shp)
                # in1[.., j, i, m] = rhs[.., m, i] -> transpose to [.., i, m]
                perm = list(range(len(rhs_mi.shape)))
                perm[-2], perm[-1] = perm[-1], perm[-2]
                in1 = rhs_mi.transpose(perm).unsqueeze(nd - 3).to_broadcast(shp)
                nc.vector.tensor_tensor(t4, in0, in1, OP.add)
                nc.vector.reduce_max(out_ji, t4, axis=AX.X)

            # ---------- Stage A ----------
            sA = nc.named_scope("stageA")
            sA.__enter__()
            for g in range(NG):
                gt = gtiles[g]
                gv = gt.rearrange("p (ch n) -> p ch n", ch=HCH)
                eT_ps = ps_eT.tile([48, UG * 128], F32, tag="eT")
                # seed: trans+b as fp16 hi+lo rows against a ones stream
                nc.tensor.matmul(eT_ps, wt[0:2, HCH * 48:HCH * 48 + 48], ones,
                                 start=True, stop=False)
                for ch in range(HCH):
                    nc.tensor.matmul(eT_ps, wt[:, ch * 48:(ch + 1) * 48],
                                     gv[:, ch, :], start=False,
                                     stop=(ch == HCH - 1))
                eT_sb = et_pool.tile([48, UG * 128], F32, tag="eTsb")
                nc.scalar.copy(eT_sb, eT_ps)
                fx_ps = ps_fx.tile([128, UG * 48], F32, tag="fx")
                for uu in range(UG):
                    nc.tensor.transpose(fx_ps[:, uu * 48:(uu + 1) * 48],
                                        eT_sb[:, uu * 128:(uu + 1) * 128], id48)
                fx_sb = et_pool.tile([128, UG * 48], F32, tag="fxsb")
                nc.scalar.copy(fx_sb, fx_ps)
                fxv = fx_sb.rearrange("p (u c) -> p u c", u=UG)
                # TS[g] = hi + lo
                nc.vector.tensor_tensor(TS[:, g, :, :], fxv[:, :, 0:16],
                                        fxv[:, :, 32:48], OP.add)
                if g == 0:
                    # chunk-0 start fix (no-op +0.0 on rows c!=0)
                    nc.vector.tensor_tensor(TS[:, 0, 0, :], TS[:, 0, 0, :],
                                            rcfix, OP.add)
                # ---- intra-group prefix chain (transposed storage)
                tsv = TS[:, g, :, :].rearrange("p u (j m) -> p u j m", j=4)
                for uu in range(1, UG):
                    prev = (tsv[:, 0, :, :] if uu == 1 else
                            PT[:, g, uu - 2, :].rearrange("p (m i) -> p m i", m=4))
                    # PT[g,uu-1][j][i] = max_m TS[g,uu][j][m] + prev[m][i]
                    mm_compose(
                        PT[:, g, uu - 1, :].rearrange("p (j i) -> p j i", j=4),
                        tsv[:, uu, :, :], prev, gtmp_pool, "pchain")
                # ---- intra-group exclusive suffix chain (natural storage)
                # SN[g,2] = T_{g,3} natural = TS[g,3] transposed (Scalar copy)
                nc.scalar.copy(
                    SN[:, g, 2, :].rearrange("p (i x) -> p i x", i=4),
                    tsv[:, 3, :, :].transpose([0, 2, 1]))
                for s in (1, 0):
                    # SN[g,s][i][x] = max_m T_{g,s+1}[i][m] + SN[g,s+1][m][x]
                    #              = max_m TS[g,s+1][m][i] + SN[g,s+1][m][x]
                    mm_compose(
                        SN[:, g, s, :].rearrange("p (i x) -> p i x", i=4),
                        tsv[:, s + 1, :, :].transpose([0, 2, 1]),
                        SN[:, g, s + 1, :].rearrange("p (m x) -> p m x", m=4),
                        gtmp_pool, "schain")
                # ---- running cross-group prefix (transposed storage)
                # GpreT[g+1][j][i] = max_m F_g[m][j]... = max_m PT_F[j][m] + GpreT[g][m][i]
                mm_compose(
                    GpreT[:, g + 1, :].rearrange("p (j i) -> p j i", j=4),
                    PT[:, g, 2, :].rearrange("p (j m) -> p j m", j=4),
                    GpreT[:, g, :].rearrange("p (m i) -> p m i", m=4),
                    tmp_pool, "gpre")
            sA.__exit__(None, None, None)

            # ---------- Gsuf: exclusive suffix-of-groups (tail, serial) ----
            _sg = nc.named_scope("gsuf")
            _sg.__enter__()
            for g in range(NG - 2, -1, -1):
                # GsufN[g][i][x] = max_m F_{g+1}[i][m] + GsufN[g+1][m][x]
                #   F_{g+1}[i][m] = PT[g+1,2][m][i]
                mm_compose(
                    GsufN[:, g, :].rearrange("p (i x) -> p i x", i=4),
                    PT[:, g + 1, 2, :].rearrange("p (m i) -> p m i", m=4)
                        .transpose([0, 2, 1]),
                    GsufN[:, g + 1, :].rearrange("p (m x) -> p m x", m=4),
                    tmp_pool, "gsuf")
            _sg.__exit__(None, None, None)

            # ---------- p2: per-sequence boundary scans over chunks ----------
            _sp2 = nc.named_scope("p2")
            _sp2.__enter__()
            # M_c transposed [j*4+i] by (b, c) rows -> ab2[0]
            nc.sync.dma_start(
                ab2[:, 0, :, :].rearrange("p c x -> p (c x)"), GpreT[:, NG, :])
            # natural form for the bwd scan (Scalar, overlaps Gsuf)
            nc.scalar.copy(
                ab2[:, 1, :, :].rearrange("p c (i j) -> p c i j", i=4),
                ab2[:, 0, :, :].rearrange("p c (j i) -> p c j i", j=4)
                    .transpose([0, 1, 3, 2]))
            a2f = ab2[:, 0, :, :].rearrange("p c (j i) -> p c j i", j=4)
            a2b = ab2[:, 1, :, :].rearrange("p c (i j) -> p c i j", i=4)
            stv = states
            for s in range(C - 1):
                # fwd: s_{s+1}[j] = max_i s_s[i] + M_s[i][j]
                tf = gtmp_pool.tile([BC, 4, 4], F32, tag="p2f")
                nc.vector.tensor_tensor(
                    tf,
                    stv[:, 0, s, :].unsqueeze(1).to_broadcast((BC, 4, 4)),
                    a2f[:, s, :, :], OP.add)
                nc.vector.reduce_max(stv[:, 0, s + 1, :], tf, axis=AX.X)
                # bwd: c = C-1-s: bb_{c-1}[i] = max_j M_c[i][j] + bb_c[j]
                cc = C - 1 - s
                tb = gtmp_pool.tile([BC, 4, 4], F32, tag="p2b")
                nc.vector.tensor_tensor(
                    tb,
                    stv[:, 1, cc, :].unsqueeze(1).to_broadcast((BC, 4, 4)),
                    a2b[:, cc, :, :], OP.add)
                nc.vector.reduce_max(stv[:, 1, cc - 1, :], tb, axis=AX.X)
            # last fwd boundary s_16 is unused (only s_0..s_15 feed chunks)
            # broadcast boundary vectors back to (b, c) rows
            nc.sync.dma_start(
                sbc[:, 0:4], states[:, 0, 0:C, :].rearrange("p c x -> p (c x)"))
            nc.scalar.dma_start(
                sbc[:, 4:8], states[:, 1, 0:C, :].rearrange("p c x -> p (c x)"))
            _sp2.__exit__(None, None, None)

            # ---------- boundary -> group -> step backfill ----------
            _sp3 = nc.named_scope("p3")
            _sp3.__enter__()
            # asb[g][j] = max_i sb[i] + Gpre[g][i][j] = max_i GpreT[g][j][i]+sb[i]
            t4 = tmp_pool.tile([128, NG, 4, 4], F32, tag="asb")
            nc.vector.tensor_tensor(
                t4,
                GpreT[:, 0:NG, :].rearrange("p g (j i) -> p g j i", j=4),
                sbc[:, 0:4].unsqueeze(1).unsqueeze(1).to_broadcast((128, NG, 4, 4)),
                OP.add)
            nc.vector.reduce_max(ab[:, 0, :, :], t4, axis=AX.X)
            # bsb[g][i] = max_x GsufN[g][i][x] + bb[x]
            t4 = tmp_pool.tile([128, NG, 4, 4], F32, tag="bsb")
            nc.vector.tensor_tensor(
                t4,
                GsufN.rearrange("p g (i x) -> p g i x", i=4),
                sbc[:, 4:8].unsqueeze(1).unsqueeze(1).to_broadcast((128, NG, 4, 4)),
                OP.add)
            nc.vector.reduce_max(ab[:, 1, :, :], t4, axis=AX.X)
            # alpha[(g,0)][j] = max_i asb[g][i] + TS[g,0][j][i]
            t4 = tmp_pool.tile([128, NG, 4, 4], F32, tag="al0")
            nc.vector.tensor_tensor(
                t4,
                TS[:, :, 0, :].rearrange("p g (j i) -> p g j i", j=4),
                ab[:, 0, :, :].unsqueeze(2).to_broadcast((128, NG, 4, 4)),
                OP.add)
            nc.vector.reduce_max(alpha[:, :, 0, :], t4, axis=AX.X)
            # alpha[(g,uu)][j] = max_i asb[g][i] + PT[g,uu-1][j][i], uu=1..3
            # (s,j) merged: ISA allows at most 3 free dims per AP
            t5 = tmp_pool.tile([128, NG, 12, 4], F32, tag="al")
            nc.vector.tensor_tensor(
                t5,
                PT.rearrange("p g s (j i) -> p g (s j) i", j=4),
                ab[:, 0, :, :].unsqueeze(2).to_broadcast((128, NG, 12, 4)),
                OP.add)
            nc.vector.reduce_max(
                alpha[:, :, 1:UG, :].rearrange("p g u j -> p g (u j)"),
                t5, axis=AX.X)
            # beta[(g,uu)][i] = max_x SN[g,uu][i][x] + bsb[g][x], uu=0..2
            t5 = tmp_pool.tile([128, NG, 12, 4], F32, tag="be")
            nc.vector.tensor_tensor(
                t5,
                SN.rearrange("p g s (i x) -> p g (s i) x", i=4),
                ab[:, 1, :, :].unsqueeze(2).to_broadcast((128, NG, 12, 4)),
                OP.add)
            nc.vector.reduce_max(
                beta[:, :, 0:3, :].rearrange("p g u i -> p g (u i)"),
                t5, axis=AX.X)
            # beta[(g,3)] = bsb[g]
            nc.scalar.copy(beta[:, :, 3, :], ab[:, 1, :, :])
            _sp3.__exit__(None, None, None)

            # ---------- tags: first-argmax_j(alpha+beta) ----------
            _sp5 = nc.named_scope("p5")
            _sp5.__enter__()
            delta = singles.tile([128, L, 4], F32)
            av = alpha.rearrange("p g u j -> p (g u) j")
            bv = beta.rearrange("p g u j -> p (g u) j")
            nc.vector.tensor_tensor(delta, av, bv, OP.add)
            mx = tmp_pool.tile([128, L], F32, tag="mx")
            nc.vector.reduce_max(mx, delta, axis=AX.X)
            eq = singles.tile([128, L, 4], F32)
            nc.vector.tensor_tensor(
                eq, delta, mx.unsqueeze(2).to_broadcast((128, L, 4)), OP.is_equal)
            nc.vector.tensor_tensor(
                eq, eq, wfirst.unsqueeze(1).to_broadcast((128, L, 4)), OP.mult)
            nc.vector.reduce_max(mx, eq, axis=AX.X)
            nc.vector.tensor_tensor(
                eq, eq, mx.unsqueeze(2).to_broadcast((128, L, 4)), OP.is_equal)
            nc.vector.tensor_tensor(
                eq, eq, iw4.unsqueeze(1).to_broadcast((128, L, 4)), OP.mult)
            tagf = tmp_pool.tile([128, L], F32, tag="tagf")
            nc.vector.reduce_sum(tagf, eq, axis=AX.X)
            tagi = tmp_pool.tile([128, L], I32, tag="tagi")
            nc.scalar.copy(tagi, tagf)
            nc.scalar.dma_start(tagsd[:].rearrange("b (c t) -> b c t", c=C), tagi)
            _sp5.__exit__(None, None, None)

    return nc


def _split_multi_waits(nc, waits_per_drain=1):
    """Walrus (bass2jax path) allows very few embedded sync waits per
    instruction (PE matmul: exactly 1). Hoist multi-waits onto standalone
    InstDrain instructions on the same engine, preserving order."""
    for f in nc.m.functions:
        for blk in f.blocks:
            insts = blk.instructions
            i = 0
            while i < len(insts):
                ins = insts[i]
                si = ins.sync_info
                w = list(si.on_wait) if (si is not None and si.on_wait) else []
                if len(w) >= 2:
                    groups = [w[j:j + waits_per_drain]
                              for j in range(0, len(w), waits_per_drain)]
                    for k, grp in enumerate(groups):
                        d = mybir.InstEventSemaphore(
                            name=nc.get_next_instruction_name(), ins=[], outs=[])
                        d.engine = ins.engine
                        d.sync_info = mybir.SyncInfo(on_wait=grp, on_update=[])
                        insts.insert(i + k, d)
                    i += len(groups)
                    ins.sync_info = mybir.SyncInfo(
                        on_wait=[], on_update=list(si.on_update or []))
                i += 1


def _get_nc():
    if "nc" not in _NC_CACHE:
        nc = build_nc()
        _split_multi_waits(nc, waits_per_drain=1)   # HW path only
        _NC_CACHE["nc"] = nc
    return _NC_CACHE["nc"]


def make_in_maps(inputs):
    sent = np.ascontiguousarray(np.asarray(inputs["sentences"], dtype=np.float32))
    W = np.ascontiguousarray(np.asarray(inputs["W"], dtype=np.float32))
    bb = np.asarray(inputs["b"], dtype=np.float32)
    st = np.asarray(inputs["start_transitions"], dtype=np.float32)
    en = np.asarray(inputs["end_transitions"], dtype=np.float32)
    tr = np.asarray(inputs["transitions"], dtype=np.float32)

    # W' lhsT: per h-chunk 48 cols: 0:16 = Whi[j] repl. over k, 32:48 = Wlo
    wT = np.transpose(W.reshape(K, HCH, 128), (2, 1, 0))   # [p, ch, j]
    whi = wT.astype(np.float16).astype(np.float32)
    wlo = (wT - whi).astype(np.float16).astype(np.float32)
    wt = np.zeros((128, HCH + 1, 48), dtype=np.float32)
    wt[:, :HCH, 0:16] = np.repeat(whi, 4, axis=2)
    wt[:, :HCH, 32:48] = np.repeat(wlo, 4, axis=2)
    # seed block: rows 0/1 = tbT hi/lo at cols 0:16
    tbT = (tr.T + bb[:, None]).reshape(16).astype(np.float32)
    tbhi = tbT.astype(np.float16).astype(np.float32)
    wt[0, HCH, 0:16] = tbhi
    wt[1, HCH, 0:16] = tbT - tbhi
    wt = np.ascontiguousarray(wt.reshape(128, WCOLS)).astype(np.float16)

    mpid = (np.where(np.eye(4, dtype=bool), 0.0, NEG)).astype(np.float32).ravel()
    fix = (st[:, None] - tr.T).reshape(16).astype(np.float32)

    rcm = np.zeros((128, 128), dtype=np.float32)
    rcm[:, 0:4] = [4.0, 3.0, 2.0, 1.0]
    rcm[:, 4:8] = [0.0, 1.0, 2.0, 3.0]
    rcm[:, 8:24] = mpid[None, :]
    rcm[:, 24:28] = en[None, :]
    rcm[0::C, 28:44] = fix[None, :]
    rcm[0:48, 44:92] = np.eye(48, dtype=np.float32)

    in_maps = []
    for core in range(NCORES):
        sc = sent[core * BC:(core + 1) * BC]           # [8, 512, 768]
        s6 = sc.reshape(BC, C, NG, UG, HCH, 128)       # b c g uu ch p
        sT = np.transpose(s6, (5, 2, 4, 3, 0, 1))      # p g ch uu b c
        sT = np.ascontiguousarray(sT.reshape(128, NG * GW)).astype(np.float16)
        in_maps.append({
            "sentT": sT, "wt": wt, "rowconsts": rcm,
        })
    return in_maps


def kernel(**inputs):
    nc = _get_nc()
    in_maps = make_in_maps(inputs)
    res = run_bass_kernel_spmd(nc, in_maps, core_ids=list(range(NCORES)))
    tags = np.concatenate([res.results[c]["tags"] for c in range(NCORES)], axis=0)
    return tags.astype(np.int32)


if __name__ == "__main__":
    import reference
    inputs = {k: np.asarray(v) for k, v in reference.setup_inputs().items()}
    out = kernel(**inputs)
    print(out.shape, out.dtype, out[:2, :16])
